# revision 111
# baseline (speedup 1.0000x reference)
"""Trainium2 Bass kernel for the Chebyshev atomic descriptor (gnn_message_passing).

Contract: kernel(**inputs) takes FULL unsharded inputs (positions [20000,3] f32,
species_idx [20000] i32, neighbor_idx [480000] i32) and returns the full
[20000, 52] f32 feature array. Sharding: data-parallel over atoms across 8
NeuronCores; each core receives its atoms' K neighbor slots as dense position/
spin planes (host-side input distribution per the sharding contract), so the
device does dense loads instead of 256B-granularity random gathers.

Algorithm (same math as the proven baseline): angular (triplet) features via
the spherical-harmonic addition theorem:
  sum_{j<k} w_j w_k T_t(u_j.u_k) = 1/2 (sum_l lam_{t,l} Q_l - F2),
  Q_l = sum_m gamma_lm B_lm^2,  B_lm = sum_j w_j Ybar_lm(u_j),  F2 = sum_j w_j^2
with real solid harmonics per neighbor: sectoral values by a Chebyshev-style
three-term recurrence on (rho^m cos/sin m phi), associated-Legendre z-ladder,
and per-(m,t) weight products, all in fp16. All K-reductions (radial Chebyshev
chains, angular moments, Q_l segment sums) run on the TensorEngine as
identity-stationary accumulating matmuls over a PACKED 81-component layout
(no zero columns). Radial Chebyshev uses a step-2 recurrence with both spin
chains interleaved.
"""

import math
from contextlib import ExitStack

import numpy as np

import bass_rust
import concourse.bass as bass
import concourse.bacc as bacc
import concourse.tile as tile
from concourse import mybir
from concourse.bass_utils import run_bass_kernel_spmd

F32 = mybir.dt.float32
F16 = mybir.dt.float16
Alu = mybir.AluOpType
Act = mybir.ActivationFunctionType
AX = mybir.AxisListType

# ---- problem constants (hardcoded per harness contract) ----
N = 20000
K = 24
NCORES = 8
NPAD = 20480
NPC = NPAD // NCORES     # atoms per core = 2560
PT = 128                 # partitions
G = 10                   # atoms per partition per supertile
SUP = NPC // (PT * G)    # supertiles per core = 2
STA = PT * G             # atoms per supertile = 1280
SLOT = G * K             # neighbor slots per partition per supertile = 240
RAD_ORDER = 16
ANG_ORDER = 8
L = ANG_ORDER
NRAD = RAD_ORDER + 1     # 17
NANG = ANG_ORDER + 1     # 9
RAD_CUT = 8.0
ANG_CUT = 6.5
MIN_CUT = 0.55
FEAT = 52
NCH_RAD = 2 * NRAD + 1   # 34 interleaved (t,chain) + F2 = 35
PACK = 81                # packed angular comps
# PE matmul groups on m-block boundaries so matmuls start as soon as the
# group's MP products land; the LAST group is tiny so the per-supertile tail
# (last group's matmuls -> SQ -> gamma -> Q) is short: 39, 36, 6 comps
GRP_MS = [(0, 1, 2), (3, 4, 5), (6, 7, 8)]
NGRP = len(GRP_MS)

HALF_PI = math.pi / 2.0
AX_ = 2.0 / (RAD_CUT - MIN_CUT)
BX_ = -2.0 * MIN_CUT / (RAD_CUT - MIN_CUT) - 1.0

# packed m-major comp order: m=0 -> l=0..8 (t=0 only); m>=1 -> t in {0,1},
# l = m..8 within each t block.  PB[m] = packed base index of m's block.
PB = [0]
for _m in range(1, 10):
    PB.append(PB[-1] + (9 - _m + 1) * (1 if _m == 1 else 2))
# PB = [0, 9, 25, 39, 51, 61, 69, 75, 79, 81]
GRP_OFF = [PB[ms[0]] for ms in GRP_MS]                  # packed offset per group
GRP_SZ = [PB[ms[-1] + 1] - PB[ms[0]] for ms in GRP_MS]  # 25, 26, 30


# ---------------------------------------------------------------------------
# host-side constant tables (ladder recurrence + quadratic-form weights)
# ---------------------------------------------------------------------------
def _dfact(n):
    r = 1
    while n > 1:
        r *= n
        n -= 2
    return r


def _a_norm(l, m):
    if m == 0:
        return 1.0
    return math.sqrt(2.0 * math.factorial(l - m) / math.factorial(l + m))


def _ladder_coeffs():
    """Monic z-ladder: A~_m = 1, A~_{m+1} = z, A~_l = z A~_{l-1} + gt A~_{l-2};
    Ybar_lm = sig_lm * A~_lm * trig_m. Returns gt[(l,m)], sig[(l,m)]."""
    gt, sig = {}, {}
    for m in range(L + 1):
        k = {m: 1.0 / _dfact(2 * m - 1)}
        if m + 1 <= L:
            k[m + 1] = k[m] / (2 * m + 1)
        for l in range(m + 2, L + 1):
            beta = (2 * l - 1) / (l - m)
            gam = -(l + m - 1) / (l - m)
            k[l] = k[l - 1] / beta
            gt[(l, m)] = gam * k[l] / k[l - 2]
        for l in range(m, L + 1):
            sig[(l, m)] = _a_norm(l, m) / k[l]
    return gt, sig


def _cheb_to_legendre():
    from numpy.polynomial import legendre as npleg, chebyshev as npcheb

    lam = np.zeros((NANG, L + 1))
    for t in range(NANG):
        c = np.zeros(t + 1)
        c[t] = 1.0
        lam[t, : t + 1] = npleg.poly2leg(npcheb.cheb2poly(c))[: t + 1]
    return lam


LAM = _cheb_to_legendre()
GT, SIG = _ladder_coeffs()


def _const_tables():
    # ccoef f16 [81]: gt at slot l*9+m (l-major rect), 0 elsewhere
    ccoef = np.zeros(81, np.float16)
    for (l, m), v in GT.items():
        ccoef[l * 9 + m] = np.float16(v)
    # gamp f16 [81]: sig^2 at PACKED slot
    gamp = np.zeros(PACK, np.float16)
    for m in range(L + 1):
        nt = 1 if m == 0 else 2
        nl = 9 - m
        for t in range(nt):
            for li in range(nl):
                l = m + li
                gamp[PB[m] + t * nl + li] = np.float16(SIG[(l, m)] ** 2)
    # lamt f16 [9 x 10]: 0.5*lam[t,l] for l<=8, -0.5 at l-slot 9 (F2)
    lamt = np.zeros(9 * 10, np.float16)
    for t in range(NANG):
        for l in range(9):
            lamt[t * 10 + l] = np.float16(0.5 * LAM[t, l])
        lamt[t * 10 + 9] = np.float16(-0.5)
    ident = np.eye(PT, dtype=np.float16)
    return ccoef, gamp, lamt, ident


def view(ap, off, dims):
    """Free-dim view of a tile AP: keep the partition entry, replace free dims
    with explicit [step, count] pairs, shift the element offset by `off`."""
    base = list(ap.ap[0])
    return bass_rust.AP(ap.tensor, ap.offset + off, [base] + [list(d) for d in dims])


def build_supertile(nc, tl, s, mix_prev=None, mix_post_prev=None, final=False):
    """Emit one supertile's compute. tl = dict of persistent tiles."""
    b = s % 2
    pn = tl[f"pn{b}"]
    ns = tl[f"ns{b}"]
    ps = tl[f"ps{b}"]
    TT = nc.vector.tensor_tensor
    TS = nc.vector.tensor_scalar
    CP = nc.vector.tensor_copy

    # ---------------- prep: distances, masks, weights (f32 -> f16) ---------
    r012 = tl["r012"]
    TT(out=r012[:], in0=pn[:],
       in1=view(ps[:], 0, [[G, 3], [0, K], [1, G]]), op=Alu.subtract)
    sq012 = tl["sq012"]
    nc.scalar.activation(sq012[:], r012[:], Act.Square)
    d2 = tl["d2"]
    TT(out=d2[:], in0=view(sq012[:], 0, [[1, SLOT]]),
       in1=view(sq012[:], SLOT, [[1, SLOT]]), op=Alu.add)
    TT(out=d2[:], in0=d2[:], in1=view(sq012[:], 2 * SLOT, [[1, SLOT]]), op=Alu.add)
    TS(out=d2[:], in0=d2[:], scalar1=1e-18, scalar2=None, op0=Alu.max)
    dd = tl["dd"]
    nc.scalar.sqrt(dd[:], d2[:])
    rinv = tl["rinv"]
    nc.vector.reciprocal(rinv[:], dd[:])
    # unit vector -> ANG channels (uz, C1, S1); host plane order is (z, x, y)
    ANG = tl["ANG"]
    TT(out=view(ANG[:], 0, [[SLOT, 3], [1, SLOT]]), in0=r012[:],
       in1=view(rinv[:], 0, [[0, 3], [1, SLOT]]), op=Alu.mult)
    # m2 mask from f32 d (reference-exact boundary at MIN_CUT)
    m2h = tl["m2h"]
    TS(out=m2h[:], in0=dd[:], scalar1=MIN_CUT, scalar2=None, op0=Alu.is_gt)
    d16 = tl["d16"]
    nc.scalar.copy(out=d16[:], in_=dd[:])
    # clamped distances (radial ch0, angular ch1) and cosine-cutoff sines
    dc2 = tl["dc2"]
    TS(out=view(dc2[:], 0, [[1, SLOT]]), in0=d16[:], scalar1=RAD_CUT,
       scalar2=None, op0=Alu.min)
    TS(out=view(dc2[:], SLOT, [[1, SLOT]]), in0=d16[:], scalar1=ANG_CUT,
       scalar2=None, op0=Alu.min)
    grad2 = tl["grad2"]
    half_pi = tl["half_pi"]
    nc.scalar.activation(view(grad2[:], 0, [[1, SLOT]]),
                         view(dc2[:], 0, [[1, SLOT]]),
                         Act.Sin, bias=half_pi[:], scale=-math.pi / RAD_CUT)
    nc.scalar.activation(view(grad2[:], SLOT, [[1, SLOT]]),
                         view(dc2[:], SLOT, [[1, SLOT]]),
                         Act.Sin, bias=half_pi[:], scale=-math.pi / ANG_CUT)
    # gp = 0.5*(sin+1); the outer-cutoff mask is implied by the clamp
    # (dc2 = min(d, cut) makes gp exactly 0 beyond the cutoff), so only the
    # inner d > MIN_CUT mask applies: w = gp * m2 -> (wr, wa)
    TS(out=grad2[:], in0=grad2[:], scalar1=1.0, scalar2=0.5, op0=Alu.add,
       op1=Alu.mult)
    w2 = tl["w2"]
    TT(out=w2[:], in0=grad2[:], in1=view(m2h[:], 0, [[0, 2], [1, SLOT]]),
       op=Alu.mult)
    wr_v = view(w2[:], 0, [[1, SLOT]])
    wa_v = view(w2[:], SLOT, [[1, SLOT]])

    # ---------------- radial chains (step-2 Chebyshev, chains interleaved) --
    Srad = tl["Srad"]
    # seed S0 = wr; chain B is one exact multiply by sn (=+-1) at the end
    CP(out=view(Srad[:], 0, [[1, SLOT]]), in_=wr_v)
    # F2 = wa^2 at channel 34
    TT(out=view(Srad[:], 34 * SLOT, [[1, SLOT]]), in0=wa_v, in1=wa_v,
       op=Alu.mult)
    # x maps
    xx2 = tl["xx2"]
    TS(out=view(xx2[:], 0, [[1, SLOT]]), in0=d16[:], scalar1=AX_, scalar2=BX_,
       op0=Alu.mult, op1=Alu.add)
    # S1 = x * S0 (chain A only)
    TT(out=view(Srad[:], 2 * SLOT, [[1, SLOT]]),
       in0=view(Srad[:], 0, [[1, SLOT]]),
       in1=view(xx2[:], 0, [[1, SLOT]]), op=Alu.mult)
    # y = T2 = 2x^2 - 1 ; y2 = 2T2 - 1 ; y3 = 2T2
    yt = tl["yt"]
    TT(out=view(yt[:], 0, [[1, SLOT]]), in0=view(xx2[:], 0, [[1, SLOT]]),
       in1=view(xx2[:], 0, [[1, SLOT]]), op=Alu.mult)
    TS(out=view(yt[:], 0, [[1, SLOT]]), in0=view(yt[:], 0, [[1, SLOT]]),
       scalar1=2.0, scalar2=-1.0, op0=Alu.mult, op1=Alu.add)
    TS(out=view(yt[:], SLOT, [[1, SLOT]]), in0=view(yt[:], 0, [[1, SLOT]]),
       scalar1=2.0, scalar2=-1.0, op0=Alu.mult, op1=Alu.add)
    TS(out=view(yt[:], 2 * SLOT, [[1, SLOT]]), in0=view(yt[:], 0, [[1, SLOT]]),
       scalar1=2.0, scalar2=None, op0=Alu.mult)
    # S2 = T2 * S0 ; S3 = (2T2-1) * S1
    TT(out=view(Srad[:], 4 * SLOT, [[1, SLOT]]),
       in0=view(Srad[:], 0, [[1, SLOT]]),
       in1=view(yt[:], 0, [[1, SLOT]]), op=Alu.mult)
    TT(out=view(Srad[:], 6 * SLOT, [[1, SLOT]]),
       in0=view(Srad[:], 2 * SLOT, [[1, SLOT]]),
       in1=view(yt[:], SLOT, [[1, SLOT]]), op=Alu.mult)
    # double-steps: (S_t, S_{t+1}) = 2T2*(S_{t-2},S_{t-1}) - (S_{t-4},S_{t-3})
    rt4 = tl["rt4"]
    y3_2 = view(yt[:], 2 * SLOT, [[0, 2], [1, SLOT]])
    for t in range(4, 16, 2):
        rt = view(rt4[:], 0, [[SLOT, 2], [1, SLOT]])
        TT(out=rt, in0=view(Srad[:], 2 * (t - 2) * SLOT,
                            [[2 * SLOT, 2], [1, SLOT]]), in1=y3_2, op=Alu.mult)
        TT(out=view(Srad[:], 2 * t * SLOT, [[2 * SLOT, 2], [1, SLOT]]),
           in0=rt, in1=view(Srad[:], 2 * (t - 4) * SLOT,
                            [[2 * SLOT, 2], [1, SLOT]]), op=Alu.subtract)
    # final single t=16
    TT(out=view(rt4[:], 0, [[1, SLOT]]),
       in0=view(Srad[:], 2 * 14 * SLOT, [[1, SLOT]]),
       in1=view(yt[:], 2 * SLOT, [[1, SLOT]]), op=Alu.mult)
    TT(out=view(Srad[:], 2 * 16 * SLOT, [[1, SLOT]]),
       in0=view(rt4[:], 0, [[1, SLOT]]),
       in1=view(Srad[:], 2 * 12 * SLOT, [[1, SLOT]]), op=Alu.subtract)
    # chain B: odd channels = even channels * sn, one exact multiply
    TT(out=view(Srad[:], SLOT, [[2 * SLOT, NRAD], [1, SLOT]]),
       in0=view(Srad[:], 0, [[2 * SLOT, NRAD], [1, SLOT]]),
       in1=view(ns[:], 0, [[0, NRAD], [1, SLOT]]), op=Alu.mult)

    # previous supertile's mix first: its accR/acc readers must be emitted
    # before this supertile's matmuls overwrite those PSUM regions
    if mix_prev is not None:
        mix_prev()
        mix_post_prev()

    ident = tl["ident"]
    accR = tl["accR"]
    fo = (s % 2) * G * FEAT
    featt, Qs = tl["featt"], tl["Qs"]


    # ---------------- sectoral recurrence (rho^m cos/sin m phi) ------------
    # a = 2ux, b = rho^2 = 1 - uz^2 (b2 ch1 is a persistent zero channel)
    a_ = tl["a_"]
    TS(out=a_[:], in0=view(ANG[:], SLOT, [[1, SLOT]]), scalar1=2.0,
       scalar2=None, op0=Alu.mult)
    b2 = tl["b2"]
    TT(out=view(b2[:], 0, [[1, SLOT]]), in0=view(ANG[:], 0, [[1, SLOT]]),
       in1=view(ANG[:], 0, [[1, SLOT]]), op=Alu.mult)
    TS(out=view(b2[:], 0, [[1, SLOT]]), in0=view(b2[:], 0, [[1, SLOT]]),
       scalar1=-1.0, scalar2=1.0, op0=Alu.mult, op1=Alu.add)
    st2, st2b = tl["st2"], tl["st2b"]
    a_b = view(a_[:], 0, [[0, 2], [1, SLOT]])
    b_b = view(b2[:], 0, [[0, 2], [1, SLOT]])
    for m in range(2, L + 1):
        prev = view(ANG[:], (1 + 2 * (m - 2)) * SLOT, [[SLOT, 2], [1, SLOT]])
        TT(out=st2[:], in0=prev, in1=a_b, op=Alu.mult)
        if m == 2:
            sub = b2[:]
        else:
            prev2 = view(ANG[:], (1 + 2 * (m - 3)) * SLOT,
                         [[SLOT, 2], [1, SLOT]])
            sb = view(st2b[:], (m % 2) * 2 * SLOT, [[SLOT, 2], [1, SLOT]])
            nc.gpsimd.tensor_tensor(out=sb, in0=prev2, in1=b_b, op=Alu.mult)
            sub = sb
        TT(out=view(ANG[:], (1 + 2 * (m - 1)) * SLOT, [[SLOT, 2], [1, SLOT]]),
           in0=st2[:], in1=sub, op=Alu.subtract)

    # ---------------- z-ladder (rect l-major LAD: slot (l*9+m)*SLOT) -------
    LAD = tl["LAD"]
    ccoef = tl["ccoef"]
    uz_b = lambda n: view(ANG[:], 0, [[0, n], [1, SLOT]])
    # wa is folded into the ladder seeds (linear recurrence): diag = wa,
    # l = m+1 row = wa*z; every A~ value then carries the angular weight,
    # so no separate W = wa*sec product is needed
    CP(out=view(LAD[:], 0, [[10 * SLOT, 9], [1, SLOT]]),
       in_=view(w2[:], SLOT, [[0, 9], [1, SLOT]]))
    TT(out=view(LAD[:], 9 * SLOT, [[10 * SLOT, 8], [1, SLOT]]), in0=uz_b(8),
       in1=view(w2[:], SLOT, [[0, 8], [1, SLOT]]), op=Alu.mult)
    lt = tl["lt"]
    for l in range(2, L + 1):
        nm = l - 1  # m = 0..l-2
        TT(out=view(LAD[:], l * 9 * SLOT, [[SLOT, nm], [1, SLOT]]),
           in0=view(LAD[:], (l - 1) * 9 * SLOT, [[SLOT, nm], [1, SLOT]]),
           in1=uz_b(nm), op=Alu.mult)
        lt_v = view(lt[:], (l % 2) * 7 * SLOT, [[SLOT, nm], [1, SLOT]])
        # small-l coefficient products fit in Pool's window; big ones would
        # stall the DVE chain behind Pool's 0.42 efficiency
        lt_eng = nc.gpsimd if l <= 6 else nc.vector
        lt_eng.tensor_tensor(
            out=lt_v,
            in0=view(LAD[:], (l - 2) * 9 * SLOT, [[SLOT, nm], [1, SLOT]]),
            in1=view(ccoef[:], l * 9, [[1, nm], [0, SLOT]]), op=Alu.mult)
        TT(out=view(LAD[:], l * 9 * SLOT, [[SLOT, nm], [1, SLOT]]),
           in0=view(LAD[:], l * 9 * SLOT, [[SLOT, nm], [1, SLOT]]),
           in1=lt_v, op=Alu.add)

    # ---------------- MP products (packed m-major 81 comps) -----------------
    # emitted per PE group so chain-A matmuls start as soon as a group lands
    MPA, MPB = tl["MPA"], tl["MPB"]
    LAD = tl["LAD"]
    acc = tl["acc"]
    # radial K-reduction placed here so PE runs it back-to-back with the
    # angular groups (p-state continuity); Srad has long been ready
    for k in range(K):
        nc.tensor.matmul(
            view(accR[:], 0, [[1, NCH_RAD * G]]),
            ident[:],
            view(Srad[:], k * G, [[SLOT, NCH_RAD], [1, G]]),
            start=(k == 0),
            stop=(k == K - 1),
        )
    nc.scalar.copy(out=view(featt[:], fo + 0, [[NRAD, 2], [1, NRAD], [FEAT, G]]),
                   in_=view(accR[:], 0, [[G, 2], [2 * G, NRAD], [1, G]]))
    for gi, ms in enumerate(GRP_MS):
        for m in ms:
            nl = 9 - m
            if m == 0:
                continue  # m=0 streams straight from the weighted ladder
            # high-m blocks are small and PE-consumed (no DVE reader),
            # so Pool absorbs them without stalling the DVE chain
            eng = nc.gpsimd if m >= 5 else nc.vector
            eng.tensor_tensor(
                out=view(MPA[:], PB[m] * SLOT,
                         [[nl * SLOT, 2], [SLOT, nl], [1, SLOT]]),
                in0=view(LAD[:], (m * 9 + m) * SLOT,
                         [[0, 2], [9 * SLOT, nl], [1, SLOT]]),
                in1=view(ANG[:], (1 + 2 * (m - 1)) * SLOT,
                         [[SLOT, 2], [0, nl], [1, SLOT]]),
                op=Alu.mult)
        if gi == 0:
            # m=0: moving operand is the wa-seeded ladder itself (no MPA
            # staging); its 9*G region shares group 0's PSUM bank
            for k in range(K):
                nc.tensor.matmul(
                    view(acc[:], 0, [[1, 9 * G]]),
                    ident[:],
                    view(LAD[:], k * G, [[9 * SLOT, 9], [1, G]]),
                    start=(k == 0),
                    stop=(k == K - 1),
                )
            for k in range(K):
                nc.tensor.matmul(
                    view(acc[:], 9 * G, [[1, (GRP_SZ[0] - 9) * G]]),
                    ident[:],
                    view(MPA[:], PB[1] * SLOT + k * G,
                         [[SLOT, GRP_SZ[0] - 9], [1, G]]),
                    start=(k == 0),
                    stop=(k == K - 1),
                )
        else:
            for k in range(K):
                nc.tensor.matmul(
                    view(acc[:], gi * 512, [[1, GRP_SZ[gi] * G]]),
                    ident[:],
                    view(MPA[:], GRP_OFF[gi] * SLOT + k * G,
                         [[SLOT, GRP_SZ[gi]], [1, G]]),
                    start=(k == 0),
                    stop=(k == K - 1),
                )
    if final:
        # last supertile: chain A's mix overlaps chain B's phase
        mix_pre(nc, tl, s, 0)
    # chain B = chain A * neighbor typespin, per group so the PE tail after
    # the last DVE op is only one group's matmuls
    for gi in range(NGRP):
        if gi == 0:
            TT(out=view(MPB[:], 0, [[SLOT, 9], [1, SLOT]]),
               in0=view(LAD[:], 0, [[9 * SLOT, 9], [1, SLOT]]),
               in1=view(ns[:], 0, [[0, 9], [1, SLOT]]), op=Alu.mult)
            TT(out=view(MPB[:], 9 * SLOT, [[1, (GRP_SZ[0] - 9) * SLOT]]),
               in0=view(MPA[:], 9 * SLOT, [[1, (GRP_SZ[0] - 9) * SLOT]]),
               in1=view(ns[:], 0, [[0, GRP_SZ[0] - 9], [1, SLOT]]),
               op=Alu.mult)
        else:
            eng = nc.gpsimd if gi == NGRP - 1 else nc.vector
            eng.tensor_tensor(
               out=view(MPB[:], GRP_OFF[gi] * SLOT, [[1, GRP_SZ[gi] * SLOT]]),
               in0=view(MPA[:], GRP_OFF[gi] * SLOT, [[1, GRP_SZ[gi] * SLOT]]),
               in1=view(ns[:], 0, [[0, GRP_SZ[gi]], [1, SLOT]]), op=Alu.mult)
        for k in range(K):
            nc.tensor.matmul(
                view(acc[:], (NGRP + gi) * 512, [[1, GRP_SZ[gi] * G]]),
                ident[:],
                view(MPB[:], GRP_OFF[gi] * SLOT + k * G,
                     [[SLOT, GRP_SZ[gi]], [1, G]]),
                start=(k == 0),
                stop=(k == K - 1),
            )
    if final:
        mix_post(nc, tl, s, 0)
        mix_pre(nc, tl, s, 1)
        mix_post(nc, tl, s, 1)
        store_feat(nc, tl, s)


def store_feat(nc, tl, sp):
    nc.sync.dma_start(
        out=tl["feat_dram"][sp * STA: (sp + 1) * STA, :].rearrange(
            "(p g) f -> p (g f)", p=PT),
        in_=view(tl["featt"][:], (sp % 2) * G * FEAT, [[1, G * FEAT]]),
    )


def mix_pre(nc, tl, sp, ch):
    """One chain's PSUM evac: SQ = B^2, gamma weight, Q segment-sums on PE."""
    TT = nc.vector.tensor_tensor
    CP = nc.vector.tensor_copy
    acc = tl["acc"]
    SQ, Qp, Qs = tl["SQ"], tl["Qp"], tl["Qs"]
    ident = tl["ident"]

    if ch == 0:
        # F2 into both chains' l-slot 9 (accR of sp is long complete here)
        CP(out=view(Qs[:], (sp % 2) * 200 + 9 * G, [[10 * G, 2], [1, G]]),
           in_=view(tl["accR"][:], 34 * G, [[0, 2], [1, G]]))
    # per group: SQ = B^2 -> gamma weight -> Q_l segment sums, so each group's
    # evac pipeline fires as soon as its accumulation matmuls stop
    first = True
    for gi in range(NGRP):
        nc.scalar.activation(
            view(SQ[:], (ch * PACK + GRP_OFF[gi]) * G,
                 [[1, GRP_SZ[gi] * G]]),
            view(acc[:], (ch * NGRP + gi) * 512, [[1, GRP_SZ[gi] * G]]),
            Act.Square)
        TT(out=view(SQ[:], (ch * PACK + GRP_OFF[gi]) * G,
                    [[1, GRP_SZ[gi] * G]]),
           in0=view(SQ[:], (ch * PACK + GRP_OFF[gi]) * G,
                    [[1, GRP_SZ[gi] * G]]),
           in1=view(tl["gamp"][:], GRP_OFF[gi], [[1, GRP_SZ[gi]], [0, G]]),
           op=Alu.mult)
        for m in GRP_MS[gi]:
            nl = 9 - m
            nt = 1 if m == 0 else 2
            for t in range(nt):
                nc.tensor.matmul(
                    view(Qp[:], (ch * 10 + m) * G, [[1, nl * G]]),
                    ident[:],
                    view(SQ[:], (ch * PACK + PB[m] + t * nl) * G, [[1, nl * G]]),
                    start=first,
                    stop=(m == L and t == nt - 1),
                )
                first = False


def mix_post(nc, tl, sp, ch):
    """One chain's lambda mix: emitted well after mix_pre so the Q-matmul
    cascade latency hides behind other DVE work."""
    TT = nc.vector.tensor_tensor
    CP = nc.vector.tensor_copy
    Qp, Qs, ZT = tl["Qp"], tl["Qs"], tl["ZT"]
    featt = tl["featt"]
    fo = (sp % 2) * G * FEAT
    # Qs (f32 sbuf): l=0..8 from Qp
    qo = (sp % 2) * 200
    CP(out=view(Qs[:], qo + ch * 10 * G, [[1, 9 * G]]),
       in_=view(Qp[:], ch * 10 * G, [[1, 9 * G]]))
    # lambda mix: ang[t'] = sum_l lamt[t',l]*Qs[l] (l-slot 9 = -F2/2)
    TT(out=view(ZT[:], ch * 900, [[10 * G, 9], [10, G], [1, 10]]),
       in0=view(Qs[:], qo + ch * 10 * G, [[0, 9], [1, G], [G, 10]]),
       in1=view(tl["lamt"][:], 0, [[10, 9], [0, G], [1, 10]]), op=Alu.mult)
    nc.vector.tensor_reduce(
        out=view(featt[:], fo + 2 * NRAD + 9 * ch, [[1, 9], [FEAT, G]]),
        in_=view(ZT[:], ch * 900, [[10 * G, 9], [10, G], [1, 10]]),
        axis=AX.X, op=Alu.add)


def build_program():
    nc = bacc.Bacc("TRN2", target_bir_lowering=False, debug=False)
    pnz = nc.dram_tensor("pnz", [SUP * PT, 3 * SLOT], F32, kind="ExternalInput").ap()
    pns = nc.dram_tensor("pns", [SUP * PT, SLOT], F16, kind="ExternalInput").ap()
    psz = nc.dram_tensor("psz", [SUP * PT, 3 * G], F32, kind="ExternalInput").ap()
    ident_d = nc.dram_tensor("ident", [PT, PT], F16, kind="ExternalInput").ap()
    ccoef_d = nc.dram_tensor("ccoef", [PT, 81], F16, kind="ExternalInput").ap()
    gamp_d = nc.dram_tensor("gamp", [PT, PACK], F16, kind="ExternalInput").ap()
    lamt_d = nc.dram_tensor("lamt", [PT, 90], F16, kind="ExternalInput").ap()
    feat = nc.dram_tensor("feat", [NPC, FEAT], F32, kind="ExternalOutput").ap()

    with tile.TileContext(nc) as tc, ExitStack() as ctx:
        const = ctx.enter_context(tc.tile_pool(name="const", bufs=1))
        io = ctx.enter_context(tc.tile_pool(name="io", bufs=1))
        kp = ctx.enter_context(tc.tile_pool(name="kspace", bufs=1))
        psum = ctx.enter_context(tc.tile_pool(name="psum", bufs=1, space="PSUM"))

        tl = {}

        def T(pool, name, shape, dtype):
            tl[name] = pool.tile(shape, dtype, name=name, tag=name)
            return tl[name]

        T(const, "ident", [PT, PT], F16)
        T(const, "ccoef", [PT, 81], F16)
        T(const, "gamp", [PT, PACK], F16)
        T(const, "lamt", [PT, 90], F16)
        T(const, "half_pi", [PT, 1], F32)
        T(const, "eps_", [PT, 1], F32)

        for b in range(2):
            T(io, f"pn{b}", [PT, 3 * SLOT], F32)
            T(io, f"ns{b}", [PT, SLOT], F16)
            T(io, f"ps{b}", [PT, 3 * G], F32)

        for nm in ("r012", "sq012"):
            T(kp, nm, [PT, 3 * SLOT], F32)
        for nm in ("d2", "dd", "rinv"):
            T(kp, nm, [PT, SLOT], F32)
        for nm in ("d16", "m2h", "a_"):
            T(kp, nm, [PT, SLOT], F16)
        for nm in ("dc2", "grad2", "ml2", "mm2", "w2", "xx2", "b2", "st2"):
            T(kp, nm, [PT, 2 * SLOT], F16)
        T(kp, "st2b", [PT, 4 * SLOT], F16)
        T(kp, "yt", [PT, 3 * SLOT], F16)
        T(kp, "rt4", [PT, 4 * SLOT], F16)
        T(kp, "ANG", [PT, 17 * SLOT], F16)
        T(kp, "LAD", [PT, 81 * SLOT], F16)
        T(kp, "lt", [PT, 2 * 7 * SLOT], F16)
        T(kp, "W", [PT, 16 * SLOT], F16)
        T(kp, "MPA", [PT, PACK * SLOT], F16)
        T(kp, "MPB", [PT, PACK * SLOT], F16)
        T(kp, "Srad", [PT, NCH_RAD * SLOT], F16)
        T(kp, "SQ", [PT, 2 * PACK * G], F16)
        T(kp, "Qs", [PT, 2 * 2 * 10 * G], F16)
        T(kp, "ZT", [PT, 2 * 9 * G * 10], F16)
        T(kp, "featt", [PT, 2 * G * FEAT], F32)

        T(psum, "acc", [PT, 2 * NGRP * 512], F32)
        T(psum, "accR", [PT, 512], F32)
        T(psum, "Qp", [PT, 512], F32)

        def load(s):
            b = s % 2
            nc.sync.dma_start(
                out=tl[f"pn{b}"][:], in_=pnz[s * PT: (s + 1) * PT, :])
            nc.sync.dma_start(
                out=tl[f"ps{b}"][:], in_=psz[s * PT: (s + 1) * PT, :])
            nc.sync.dma_start(
                out=tl[f"ns{b}"][:], in_=pns[s * PT: (s + 1) * PT, :])

        load(0)
        nc.sync.dma_start(out=tl["ident"][:], in_=ident_d)
        nc.sync.dma_start(out=tl["ccoef"][:], in_=ccoef_d)
        nc.sync.dma_start(out=tl["gamp"][:], in_=gamp_d)
        nc.sync.dma_start(out=tl["lamt"][:], in_=lamt_d)
        # one-time: b2 zero channel (LAD diag is wa-seeded per supertile)
        nc.gpsimd.memset(view(tl["b2"][:], SLOT, [[1, SLOT]]), 0.0)
        nc.gpsimd.memset(tl["half_pi"][:], HALF_PI)
        nc.gpsimd.memset(tl["eps_"][:], 1e-18)

        tl["feat_dram"] = feat

        def mk_pre(sp):
            def f():
                mix_pre(nc, tl, sp, 0)
                mix_pre(nc, tl, sp, 1)
            return f

        def mk_post(sp):
            def f():
                mix_post(nc, tl, sp, 0)
                mix_post(nc, tl, sp, 1)
                store_feat(nc, tl, sp)
            return f

        for s in range(SUP):
            if s + 1 < SUP:
                load(s + 1)
            build_supertile(
                nc, tl, s,
                mix_prev=mk_pre(s - 1) if s > 0 else None,
                mix_post_prev=mk_post(s - 1) if s > 0 else None,
                final=(s == SUP - 1))

    nc.compile()
    return nc


_NC_CACHE = None


def get_program():
    global _NC_CACHE
    if _NC_CACHE is None:
        _NC_CACHE = build_program()
    return _NC_CACHE


def make_in_maps(positions, species_idx, neighbor_idx):
    pos = np.zeros((NPAD, 3), np.float32)
    pos[:N] = positions
    spin = np.zeros(NPAD, np.float16)
    spin[:N] = (2.0 * species_idx.astype(np.float32) - 1.0).astype(np.float16)
    nbrK = np.zeros((NPAD, K), np.int32)
    nbrK[:N] = neighbor_idx.reshape(N, K)

    ccoef, gamp, lamt, ident = _const_tables()
    ccoef_t = np.broadcast_to(ccoef, (PT, 81)).copy()
    gamp_t = np.broadcast_to(gamp, (PT, PACK)).copy()
    lamt_t = np.broadcast_to(lamt, (PT, 90)).copy()

    # slot = k*G + g
    sl = np.arange(SLOT)
    k_of, g_of = sl // G, sl % G
    p = np.arange(PT)
    ZXY = (2, 0, 1)  # plane order (z, x, y)

    in_maps = []
    for c in range(NCORES):
        cb = c * NPC
        pnz = np.empty((SUP * PT, 3 * SLOT), np.float32)
        pns = np.empty((SUP * PT, SLOT), np.float16)
        psz = np.empty((SUP * PT, 3 * G), np.float32)
        for s in range(SUP):
            atoms = cb + s * STA + p[:, None] * G + g_of[None, :]  # [PT, SLOT]
            nb = nbrK[atoms, k_of[None, :]]                        # [PT, SLOT]
            for ci, comp in enumerate(ZXY):
                pnz[s * PT: (s + 1) * PT, ci * SLOT: (ci + 1) * SLOT] = pos[nb, comp]
            pns[s * PT: (s + 1) * PT] = spin[nb]
            selfa = cb + s * STA + p[:, None] * G + np.arange(G)[None, :]
            for ci, comp in enumerate(ZXY):
                psz[s * PT: (s + 1) * PT, ci * G: (ci + 1) * G] = pos[selfa, comp]
        in_maps.append(
            {
                "pnz": pnz,
                "pns": pns,
                "psz": psz,
                "ident": ident,
                "ccoef": ccoef_t,
                "gamp": gamp_t,
                "lamt": lamt_t,
            }
        )
    return in_maps


def run(positions, species_idx, neighbor_idx, trace=False, trace_cores=None):
    nc = get_program()
    in_maps = make_in_maps(positions, species_idx, neighbor_idx)
    res = run_bass_kernel_spmd(
        nc,
        in_maps,
        core_ids=list(range(NCORES)),
        trace=trace,
        trace_cores=trace_cores,
    )
    out = np.concatenate([res.results[c]["feat"] for c in range(NCORES)], axis=0)
    return out[:N], res


def kernel(positions, species_idx, neighbor_idx):
    out, _ = run(positions, species_idx, neighbor_idx, trace=False)
    return out
